# revision 1
# baseline (speedup 1.0000x reference)
"""DeepseekV2 MLA attention (matrix-absorbed, causal MQA) on 8 TRN2 cores.

Tensor-parallel over heads: 2 heads per core. Per core:
  - q/kv projections from a host-pre-transposed x^T (bf16)
  - latent RMS-norm (kv_norm_w folded host-side into kc/vc)
  - scores computed in transposed layout S^T[t, s] so exp/mask/AV flow
    without transposing the attention matrix
  - softmax denominator via ones-vector matmul on PE; normalization deferred
    to the small out_v^T tensor (broadcast via gpsimd partition_broadcast)
  - o_proj over this core's 2 heads -> partial [S, HID], host sums 8 partials
"""

import os

import numpy as np
import ml_dtypes

# best-effort persistent compile cache (harmless if the PJRT plugin
# doesn't support executable serialization)
os.environ.setdefault("JAX_COMPILATION_CACHE_DIR", "/tmp/jax_cache")
os.environ.setdefault("JAX_PERSISTENT_CACHE_MIN_COMPILE_TIME_SECS", "1")

S, HID, H = 2048, 2048, 16
NOPE, ROPE, KVR, VH = 128, 64, 512, 128
SCALE = (NOPE + ROPE) ** -0.5
EPS = 1e-6
NCORES = 8
HPC = H // NCORES  # heads per core = 2

BF16 = ml_dtypes.bfloat16

_CACHE = {}


def _build_nc(s_len):
    import concourse.bass as bass
    import concourse.tile as tile
    from concourse import bacc, mybir
    from concourse.bass import ts, ds
    from concourse.masks import make_identity
    from concourse.tile_rust import add_dep_helper

    f32 = mybir.dt.float32
    bf16 = mybir.dt.bfloat16

    NB = s_len // 512   # 512-wide seq blocks
    NT = s_len // 128   # 128-wide seq tiles

    nc = bacc.Bacc()

    xt = nc.declare_dram_parameter("xt", [NB, 128, 16, 512], bf16, isOutput=False)
    wkv = nc.declare_dram_parameter("wkv", [128, 16, 576], bf16, isOutput=False)
    wqn = nc.declare_dram_parameter("wqn", [128, 16, 256], bf16, isOutput=False)
    wqp = nc.declare_dram_parameter("wqp", [128, 16, 128], bf16, isOutput=False)
    kcp = nc.declare_dram_parameter("kcp", [128, 2, 512], bf16, isOutput=False)
    vcp = nc.declare_dram_parameter("vcp", [128, 2, 4, 128], bf16, isOutput=False)
    wo = nc.declare_dram_parameter("wo", [128, 2, 2048], bf16, isOutput=False)
    # rope tables per t-tile: [p, tt, 0:64]=cos, [p, tt, 64:128]=sin_eff
    cs = nc.declare_dram_parameter("cs", [128, s_len // 128, 128], f32,
                                   isOutput=False)
    out = nc.declare_dram_parameter("out", [s_len, HID], f32, isOutput=True)

    with tile.TileContext(nc) as tc:
        with (
            tc.tile_pool(name="singles", bufs=1) as singles,
            tc.tile_pool(name="state", bufs=1) as state,
            tc.tile_pool(name="xpool", bufs=6) as xpool,
            tc.tile_pool(name="attn", bufs=17) as attnp,
            tc.tile_pool(name="work", bufs=2) as work,
            tc.tile_pool(name="scr", bufs=3) as scr,
            tc.tile_pool(name="stat", bufs=4) as statp,
            tc.tile_pool(name="outp", bufs=4) as outp,
            tc.tile_pool(name="pmm", bufs=5, space="PSUM") as pmm,
            tc.tile_pool(name="psmall", bufs=2, space="PSUM") as psmall,
            tc.tile_pool(name="pden", bufs=1, space="PSUM") as pden,
        ):
            # ---- static weights (chunked so the first kv matmuls can
            # start as soon as the first slices land) ----
            wkv_sb = singles.tile([128, 16, 576], bf16)
            for c in range(4):
                nc.sync.dma_start(out=wkv_sb[:, ts(c, 4), :],
                                  in_=wkv[:, ts(c, 4), :])
            wqn_sb = singles.tile([128, 16, 256], bf16)
            for c in range(2):
                nc.sync.dma_start(out=wqn_sb[:, ts(c, 8), :],
                                  in_=wqn[:, ts(c, 8), :])
            wqp_sb = singles.tile([128, 16, 128], bf16)
            nc.sync.dma_start(out=wqp_sb, in_=wqp[:])
            kc_sb = singles.tile([128, 2, 512], bf16)
            nc.sync.dma_start(out=kc_sb, in_=kcp[:])
            vc_sb = singles.tile([128, 2, 4, 128], bf16)
            nc.sync.dma_start(out=vc_sb, in_=vcp[:])
            wo_sb = singles.tile([128, 2, 2048], bf16)
            for c in range(2):
                nc.sync.dma_start(out=wo_sb[:, c, :], in_=wo[:, c, :])
            cs_sb = singles.tile([128, NT, 128], f32)
            nc.sync.dma_start(out=cs_sb, in_=cs[:])
            # DVE observes the cs_sb DMA via a single-wait touch op; the
            # rope muls (PSUM + cs_sb inputs) can then carry only the PE
            # wait — DVE TensorTensor encodes at most one sync wait.
            cs_touch = singles.tile([128, 1], f32)
            cs_touch_inst = nc.vector.tensor_copy(cs_touch, cs_sb[:, 0, 0:1])

            ident = singles.tile([128, 128], bf16)
            make_identity(nc, ident)
            ones_f32 = singles.tile([128, 1], f32)
            nc.vector.memset(ones_f32, 1.0)
            ones_row = singles.tile([1, 128], f32)
            nc.vector.memset(ones_row, 1.0)
            eps_sb = singles.tile([128, 1], f32)
            nc.vector.memset(eps_sb, EPS)

            # ---- running state across blocks ----
            ln_sb = state.tile([128, NT, 512], bf16)     # latent_norm [t, k]
            latT_sb = state.tile([128, 4, s_len], bf16)  # latent_norm^T [k, t]
            # k_pe_rot^T duplicated in both partition halves so each head's
            # q_pe rows (base partition 0 / 64) pair with a matching lhsT
            kpeT_sb = state.tile([128, s_len], bf16)
            ovT_sb = state.tile([128, 2, s_len], bf16)   # out_v^T (normalized)

            for j in range(NB):
                # ---- phase A: load x^T block (chunked for pipelining) ----
                # issue x chunks from gpsimd: SP's serial dma_start issue
                # (~1.6us each) otherwise delays the first kv matmuls
                xchunks = []
                for c in range(4):
                    xc = xpool.tile([128, 4, 512], bf16, tag="xt")
                    nc.gpsimd.dma_start(out=xc, in_=xt[j, :, ts(c, 4), :])
                    xchunks.append(xc)

                def xtile(kt, col0, ncol):
                    return xchunks[kt // 4][:, kt % 4, ds(col0, ncol)]

                # ---- phase B: kv projection + rmsnorm + k_pe rope ----
                pending_tpose = []
                for ttl in range(4):
                    tt = 4 * j + ttl
                    p_lat = pmm.tile([128, 512], f32, tag="mm")
                    p_pe = psmall.tile([128, 64], f32, tag="small")
                    for kt in range(16):
                        nc.tensor.matmul(
                            p_lat, lhsT=xtile(kt, ttl * 128, 128),
                            rhs=wkv_sb[:, kt, 0:512],
                            start=(kt == 0), stop=(kt == 15))
                        nc.tensor.matmul(
                            p_pe, lhsT=xtile(kt, ttl * 128, 128),
                            rhs=wkv_sb[:, kt, 512:576],
                            start=(kt == 0), stop=(kt == 15))
                    # rms over k (free dim)
                    sqs = scr.tile([128, 512], f32, tag="scr")
                    stats = statp.tile([128, 3], f32, tag="stat")
                    nc.scalar.activation(
                        out=sqs, in_=p_lat,
                        func=mybir.ActivationFunctionType.Square,
                        accum_out=stats[:, 0:1])
                    nc.scalar.activation(
                        out=stats[:, 1:2], in_=stats[:, 0:1],
                        func=mybir.ActivationFunctionType.Sqrt,
                        scale=1.0 / KVR, bias=eps_sb)
                    nc.vector.reciprocal(stats[:, 2:3], stats[:, 1:2])
                    nc.vector.tensor_scalar_mul(
                        out=ln_sb[:, tt, :], in0=p_lat, scalar1=stats[:, 2:3])
                    # k_pe rope (fp32 from psum)
                    ck_t = cs_sb[:, tt, 0:64]
                    sk_t = cs_sb[:, tt, 64:128]
                    t1 = statp.tile([128, 64], f32, tag="r1")
                    t2 = statp.tile([128, 64], f32, tag="r2")
                    i1 = nc.vector.tensor_mul(t1, p_pe, ck_t)
                    i2 = nc.vector.tensor_mul(
                        t2[:, 0:32], p_pe[:, 32:64], sk_t[:, 0:32])
                    i3 = nc.vector.tensor_mul(
                        t2[:, 32:64], p_pe[:, 0:32], sk_t[:, 32:64])
                    if j == 0:
                        for ii in (i1, i2, i3):
                            add_dep_helper(ii.ins, cs_touch_inst.ins,
                                           sync=False,
                                           reason="cs first-touch order")
                    kpe_rot = statp.tile([128, 128], bf16, tag="kprot")
                    nc.vector.tensor_add(kpe_rot[:, 0:64], t1, t2)
                    nc.vector.tensor_add(kpe_rot[:, 64:128], t1, t2)
                    pending_tpose.append((tt, kpe_rot))

                # ---- phase C: q projections for block j ----
                qls = []
                for h in range(2):
                    p_qn = pmm.tile([128, 512], f32, tag="mm")
                    for kt in range(16):
                        nc.tensor.matmul(
                            p_qn, lhsT=wqn_sb[:, kt, ts(h, 128)],
                            rhs=xtile(kt, 0, 512),
                            start=(kt == 0), stop=(kt == 15))
                    qn_bf = work.tile([128, 512], bf16, tag="qn")
                    nc.scalar.copy(qn_bf, p_qn)
                    ql = work.tile([128, 4, 512], bf16, tag=f"ql{h}")
                    for kk in range(4):
                        p_ql = pmm.tile([128, 512], f32, tag="mm")
                        nc.tensor.matmul(
                            p_ql, lhsT=kc_sb[:, h, ts(kk, 128)], rhs=qn_bf,
                            start=True, stop=True)
                        nc.vector.tensor_copy(ql[:, kk, :], p_ql)
                    qls.append(ql)
                # deferred transposes from phase B (rms/rope latency now hidden
                # behind the q projection matmuls above)
                for (tt, kpe_rot) in pending_tpose:
                    p_t = psmall.tile([128, 128], bf16, tag="small")
                    nc.tensor.transpose(p_t, kpe_rot, ident)
                    nc.scalar.copy(kpeT_sb[:, ts(tt, 128)], p_t)
                    for kk in range(4):
                        p_t2 = psmall.tile([128, 128], bf16, tag="small")
                        nc.tensor.transpose(
                            p_t2, ln_sb[:, tt, ts(kk, 128)], ident)
                        nc.vector.tensor_copy(latT_sb[:, kk, ts(tt, 128)], p_t2)

                # q_pe: untransposed per s-tile [s, d(2 heads)], rope along
                # free dim (keeps DVE inputs partition-aligned), then PE
                # transpose into [d, s] with heads stacked on partitions
                qpe_rot = work.tile([128, 512], bf16, tag="qpr")
                for stl in range(4):
                    tt = 4 * j + stl
                    p_qp = pmm.tile([128, 128], f32, tag="mm")
                    for kt in range(16):
                        nc.tensor.matmul(
                            p_qp, lhsT=xtile(kt, stl * 128, 128),
                            rhs=wqp_sb[:, kt, :],
                            start=(kt == 0), stop=(kt == 15))
                    ckq = cs_sb[:, tt, 0:64]
                    skq = cs_sb[:, tt, 64:128]
                    t1q = statp.tile([128, 128], f32, tag="qt1")
                    t2q = statp.tile([128, 128], f32, tag="qt2")
                    for h2 in range(2):
                        b = 64 * h2
                        i1 = nc.vector.tensor_mul(
                            t1q[:, b:b + 64], p_qp[:, b:b + 64], ckq)
                        i2 = nc.vector.tensor_mul(
                            t2q[:, b:b + 32], p_qp[:, b + 32:b + 64],
                            skq[:, 0:32])
                        i3 = nc.vector.tensor_mul(
                            t2q[:, b + 32:b + 64], p_qp[:, b:b + 32],
                            skq[:, 32:64])
                        if j == 0 and stl == 0:
                            for ii in (i1, i2, i3):
                                add_dep_helper(ii.ins, cs_touch_inst.ins,
                                               sync=False,
                                               reason="cs first-touch order")
                    qpr_u = statp.tile([128, 128], bf16, tag="qpru")
                    nc.vector.tensor_add(qpr_u, t1q, t2q)
                    p_tq = psmall.tile([128, 128], bf16, tag="small")
                    nc.tensor.transpose(p_tq, qpr_u, ident)
                    nc.scalar.copy(qpe_rot[:, ts(stl, 128)], p_tq)

                # ---- phase E (deferred): o_proj for the PREVIOUS block —
                # emitted before this block's ovT writes so it depends only
                # on block j-1's long-finished denominator chain ----
                def phase_e(je):
                    for stl in range(4):
                        st = 4 * je + stl
                        for hb in range(4):
                            p_o = pmm.tile([128, 512], f32, tag="mm")
                            nc.tensor.matmul(
                                p_o, lhsT=ovT_sb[:, 0, ts(st, 128)],
                                rhs=wo_sb[:, 0, ts(hb, 512)],
                                start=True, stop=False)
                            nc.tensor.matmul(
                                p_o, lhsT=ovT_sb[:, 1, ts(st, 128)],
                                rhs=wo_sb[:, 1, ts(hb, 512)],
                                start=False, stop=True)
                            ob = outp.tile([128, 512], f32, tag="ob")
                            nc.vector.tensor_copy(ob, p_o)
                            nc.sync.dma_start(
                                out=out[ts(st, 128), ts(hb, 512)], in_=ob)

                if j > 0:
                    phase_e(j - 1)

                # ---- phase D: attention for s-block j, each head ----
                n_t = 4 * (j + 1)
                for h in range(2):
                    ql = qls[h]
                    atiles = []
                    for tt in range(n_t):
                        # diagonal tiles: columns s < t are fully masked —
                        # skip them in the matmuls/exp (c0 = first live col)
                        c0 = 128 * (tt - 4 * j) if tt >= 4 * j else 0
                        p_sc = pmm.tile([128, 512], f32, tag="mm")
                        for kk in range(4):
                            nc.tensor.matmul(
                                p_sc[:, c0:512],
                                lhsT=latT_sb[:, kk, ts(tt, 128)],
                                rhs=ql[:, kk, c0:512],
                                start=(kk == 0), stop=False)
                        nc.tensor.matmul(
                            p_sc[:, c0:512],
                            lhsT=kpeT_sb[ds(64 * h, 64), ts(tt, 128)],
                            rhs=qpe_rot[ds(64 * h, 64), c0:512],
                            start=False, stop=True)
                        at = attnp.tile([128, 512], bf16, tag="attn")
                        if c0 > 0:
                            nc.gpsimd.memset(at[:, 0:c0], 0.0)
                        nc.scalar.activation(
                            out=at[:, c0:512], in_=p_sc[:, c0:512],
                            func=mybir.ActivationFunctionType.Exp, scale=SCALE)
                        if tt >= 4 * j:
                            # partial mask inside the first live 128 cols:
                            # keep where (s' - p) >= 0 in-tile
                            nc.gpsimd.affine_select(
                                out=at[:, c0:c0 + 128],
                                in_=at[:, c0:c0 + 128],
                                compare_op=mybir.AluOpType.is_ge,
                                fill=0.0, base=0,
                                channel_multiplier=-1, pattern=[[1, 128]])
                        atiles.append(at)
                    # denominator: DVE add-tree over attn tiles, then one
                    # partition-reduce matmul (cheaper on PE than n_t
                    # ones-matmuls of N cycles each)
                    dacc = scr.tile([128, 512], f32, tag="dacc")
                    nc.vector.tensor_copy(dacc, atiles[0])
                    for tt in range(1, n_t):
                        c0 = 128 * (tt - 4 * j) if tt >= 4 * j else 0
                        nc.vector.tensor_add(
                            dacc[:, c0:512], dacc[:, c0:512],
                            atiles[tt][:, c0:512])
                    p_den = pden.tile([1, 512], f32, tag="den")
                    nc.tensor.matmul(
                        p_den, lhsT=ones_f32, rhs=dacc,
                        start=True, stop=True)
                    recip = statp.tile([1, 512], f32, tag="recip")
                    nc.vector.reciprocal(recip, p_den)
                    # broadcast recip across partitions via K=1 matmul
                    p_rb = pmm.tile([128, 512], f32, tag="mm")
                    nc.tensor.matmul(
                        p_rb, lhsT=ones_row, rhs=recip, start=True, stop=True)
                    rb_sb = scr.tile([128, 512], f32, tag="scr")
                    nc.vector.tensor_copy(rb_sb, p_rb)
                    # AV: out_lat^T [k, s]
                    olT = work.tile([128, 4, 512], bf16, tag="olT")
                    for kk in range(4):
                        p_ol = pmm.tile([128, 512], f32, tag="mm")
                        for tt in range(n_t):
                            c0 = 128 * (tt - 4 * j) if tt >= 4 * j else 0
                            nc.tensor.matmul(
                                p_ol[:, c0:512],
                                lhsT=ln_sb[:, tt, ts(kk, 128)],
                                rhs=atiles[tt][:, c0:512],
                                start=(tt == 0), stop=(tt == n_t - 1))
                        nc.scalar.copy(olT[:, kk, :], p_ol)
                    # out_v^T [v, s] + deferred softmax normalization
                    p_ov = pmm.tile([128, 512], f32, tag="mm")
                    for kk in range(4):
                        nc.tensor.matmul(
                            p_ov, lhsT=vc_sb[:, h, kk, :], rhs=olT[:, kk, :],
                            start=(kk == 0), stop=(kk == 3))
                    nc.vector.tensor_mul(
                        ovT_sb[:, h, ts(j, 512)], p_ov, rb_sb)

                if j == NB - 1:
                    phase_e(j)
    nc.compile()
    return nc


def _prep_inputs(hidden_states, cos, sin, w_q, w_kv_a, kv_norm_w, kc, vc, w_o,
                 s_len):
    """Host-side sharding + layout. Returns list of 8 per-core input dicts."""
    f32 = np.float32
    x = np.asarray(hidden_states, f32)[0][:s_len]  # [S, HID]
    cos = np.asarray(cos, f32)[:s_len]
    sin = np.asarray(sin, f32)[:s_len]
    w_q = np.asarray(w_q, f32).reshape(HID, H, NOPE + ROPE)
    w_kv_a = np.asarray(w_kv_a, f32)
    w = np.asarray(kv_norm_w, f32)
    kc = np.asarray(kc, f32)
    vc = np.asarray(vc, f32)
    w_o = np.asarray(w_o, f32).reshape(H, VH, HID)

    NB = s_len // 512

    # x^T in [j, p, a, s'] blocks: x^T[hid=a*128+p, s=j*512+s']
    xT = np.ascontiguousarray(x.T).astype(BF16)    # [HID, S]
    xt_b = np.ascontiguousarray(
        xT.reshape(16, 128, NB, 512).transpose(2, 1, 0, 3))

    wkv_b = np.ascontiguousarray(
        w_kv_a.astype(BF16).reshape(16, 128, 576).transpose(1, 0, 2))

    # rope tables: sin_eff has its first half negated
    sin_eff = np.concatenate([-sin[:, :32], sin[:, 32:]], axis=1)

    in_maps = []
    for c in range(NCORES):
        hs = [HPC * c, HPC * c + 1]
        wqn_c = np.ascontiguousarray(
            w_q[:, hs, :NOPE].reshape(16, 128, 256).transpose(1, 0, 2)
        ).astype(BF16)
        wqp_c = np.ascontiguousarray(
            w_q[:, hs, NOPE:].reshape(16, 128, 128).transpose(1, 0, 2)
        ).astype(BF16)
        kc_c = np.ascontiguousarray(
            (kc[hs] * w[None, None, :]).transpose(1, 0, 2)).astype(BF16)
        vc_c = np.ascontiguousarray(
            (vc[hs] * w[None, :, None]).reshape(2, 4, 128, 128)
            .transpose(2, 0, 1, 3)).astype(BF16)
        wo_c = np.ascontiguousarray(w_o[hs].transpose(1, 0, 2)).astype(BF16)
        in_maps.append({
            "xt": xt_b, "wkv": wkv_b,
            "wqn": wqn_c, "wqp": wqp_c, "kcp": kc_c, "vcp": vc_c, "wo": wo_c,
            "cs": np.ascontiguousarray(
                np.concatenate([cos, sin_eff], axis=1)
                .reshape(s_len // 128, 128, 128).transpose(1, 0, 2)),
        })
    return in_maps


def run(inputs, trace=False, s_len=S):
    """Returns (full_output [1,S,HID] f32, exec_time_ns or None, trace_path)."""
    from concourse import bass_utils

    if s_len not in _CACHE:
        _CACHE[s_len] = _build_nc(s_len)
    nc = _CACHE[s_len]
    in_maps = _prep_inputs(**inputs, s_len=s_len)
    res = bass_utils.run_bass_kernel_spmd(
        nc, in_maps, core_ids=list(range(NCORES)), trace=False)
    acc = np.zeros((s_len, HID), np.float64)
    for r in res.results:
        acc += r["out"].astype(np.float64)
    out = acc.astype(np.float32)[None]
    return out, None, None


def _pjrt_callable(nc, n_cores):
    """Build a jax-jitted SPMD callable for `nc` (no donation: every output
    element is written by the kernel, so uninit result buffers are fine)."""
    import jax
    from jax.sharding import Mesh, PartitionSpec, NamedSharding
    from jax.experimental.shard_map import shard_map
    from concourse import bass2jax, mybir

    bass2jax.install_neuronx_cc_hook()
    part_name = nc.partition_id_tensor.name if nc.partition_id_tensor else None
    in_names, out_names, out_avals, zero_outs = [], [], [], []
    for alloc in nc.m.functions[0].allocations:
        if not isinstance(alloc, mybir.MemoryLocationSet):
            continue
        name = alloc.memorylocations[0].name
        if alloc.kind == "ExternalInput":
            if name != part_name:
                in_names.append(name)
        elif alloc.kind == "ExternalOutput":
            out_names.append(name)
            out_avals.append(jax.core.ShapedArray(
                tuple(alloc.tensor_shape), mybir.dt.np(alloc.dtype)))
            zero_outs.append(np.zeros(
                tuple(alloc.tensor_shape), mybir.dt.np(alloc.dtype)))
    n_params = len(in_names)
    all_names = in_names + out_names
    if part_name is not None:
        all_names = all_names + [part_name]

    def _body(*args):
        operands = list(args)
        if part_name is not None:
            operands.append(bass2jax.partition_id_tensor())
        outs = bass2jax._bass_exec_p.bind(
            *operands,
            out_avals=tuple(out_avals),
            in_names=tuple(all_names),
            out_names=tuple(out_names),
            lowering_input_output_aliases=(),
            sim_require_finite=True,
            sim_require_nnan=True,
            nc=nc,
        )
        return tuple(outs)

    devices = jax.devices()[:n_cores]
    mesh = Mesh(np.asarray(devices), ("core",))
    spec = PartitionSpec("core")
    donate = tuple(range(n_params, n_params + len(out_names)))
    sharded = jax.jit(
        shard_map(_body, mesh=mesh,
                  in_specs=(spec,) * (n_params + len(out_names)),
                  out_specs=(spec,) * len(out_names), check_rep=False),
        donate_argnums=donate, keep_unused=True)
    sharding = NamedSharding(mesh, spec)
    return sharded, in_names, out_names, zero_outs, sharding


def timed_run(inputs, iters=6, s_len=S):
    """Run on HW with device-resident inputs; return (out, per-call walls)."""
    import jax
    import time

    if s_len not in _CACHE:
        _CACHE[s_len] = _build_nc(s_len)
    nc = _CACHE[s_len]
    in_maps = _prep_inputs(**inputs, s_len=s_len)
    sharded, in_names, out_names, zero_outs, sharding = _pjrt_callable(
        nc, NCORES)
    concat_in = [
        jax.device_put(
            np.concatenate([np.asarray(in_maps[c][n]) for c in range(NCORES)],
                           axis=0), sharding)
        for n in in_names
    ]
    def zeros_set():
        return [
            jax.device_put(
                np.zeros((NCORES * z.shape[0], *z.shape[1:]), z.dtype),
                sharding)
            for z in zero_outs
        ]

    # donation consumes each zero set, so pre-stage one per call
    sets = [zeros_set() for _ in range(iters + 1)]
    out_arrs = jax.block_until_ready(sharded(*concat_in, *sets[0]))
    walls = []
    for it in range(iters):
        t0 = time.perf_counter()
        out_arrs = jax.block_until_ready(sharded(*concat_in, *sets[it + 1]))
        walls.append(time.perf_counter() - t0)
    full = np.asarray(out_arrs[0]).reshape(NCORES, s_len, HID)
    out = full.astype(np.float64).sum(0).astype(np.float32)[None]
    return out, walls


def async_slope(inputs, ks=(1, 9), s_len=S):
    """Dispatch K calls without blocking, block once; slope over K gives
    per-exec time with the axon RPC overhead pipelined away (if the
    device queue overlaps dispatch)."""
    import jax
    import time

    if s_len not in _CACHE:
        _CACHE[s_len] = _build_nc(s_len)
    nc = _CACHE[s_len]
    in_maps = _prep_inputs(**inputs, s_len=s_len)
    sharded, in_names, out_names, zero_outs, sharding = _pjrt_callable(
        nc, NCORES)
    concat_in = [
        jax.device_put(
            np.concatenate([np.asarray(in_maps[c][n]) for c in range(NCORES)],
                           axis=0), sharding)
        for n in in_names
    ]

    def zeros_set():
        return [
            jax.device_put(
                np.zeros((NCORES * z.shape[0], *z.shape[1:]), z.dtype),
                sharding)
            for z in zero_outs
        ]

    jax.block_until_ready(sharded(*concat_in, *zeros_set()))  # warm
    times = {}
    for k in ks:
        # ping-pong: output buffers (same shape/sharding as the donated
        # zero inputs) feed call i+2, so no host->device transfers and
        # the device queue runs the chain back-to-back
        outs = [sharded(*concat_in, *zeros_set()),
                sharded(*concat_in, *zeros_set())]
        jax.block_until_ready(outs)
        t0 = time.perf_counter()
        for i in range(k):
            outs.append(sharded(*concat_in, *outs[-2]))
        jax.block_until_ready(outs[-1])
        times[k] = time.perf_counter() - t0
    k0, k1 = ks
    slope = (times[k1] - times[k0]) / (k1 - k0)
    return slope, times


_TRIV = {}


def trivial_walls(iters=6):
    """Dispatch-overhead floor: time a near-empty 8-core bass kernel."""
    import jax
    import time
    import concourse.tile as tile
    from concourse import bacc, mybir

    if "nc" not in _TRIV:
        nc = bacc.Bacc()
        tin = nc.declare_dram_parameter("tin", [128, 128], mybir.dt.float32,
                                        isOutput=False)
        tout = nc.declare_dram_parameter("tout", [128, 128], mybir.dt.float32,
                                         isOutput=True)
        with tile.TileContext(nc) as tc:
            with tc.tile_pool(name="p", bufs=1) as p:
                t = p.tile([128, 128], mybir.dt.float32)
                nc.sync.dma_start(out=t, in_=tin[:])
                nc.sync.dma_start(out=tout[:], in_=t)
        nc.compile()
        _TRIV["nc"] = nc
    nc = _TRIV["nc"]
    sharded, in_names, out_names, zero_outs, sharding = _pjrt_callable(
        nc, NCORES)
    x = jax.device_put(np.zeros((NCORES * 128, 128), np.float32), sharding)

    def z():
        return jax.device_put(
            np.zeros((NCORES * 128, 128), np.float32), sharding)

    zs = [z() for _ in range(iters + 1)]
    jax.block_until_ready(sharded(x, zs[0]))
    walls = []
    for it in range(iters):
        t0 = time.perf_counter()
        jax.block_until_ready(sharded(x, zs[it + 1]))
        walls.append(time.perf_counter() - t0)
    return walls


def kernel(**inputs):
    out, _, _ = run(inputs, trace=False)
    return out

